# revision 14
# baseline (speedup 1.0000x reference)
"""Distributed Trainium2 kernel for pre-LN multi-head self-attention.

Reference computation (n=2048, d=1024, 16 heads x 64):
    xn  = LayerNorm(x) * ln_scale + ln_bias
    qkv = xn @ w_qkv ; split -> q,k,v [16, 2048, 64]
    sim = (q @ k^T) * d**-0.5 ; attn = softmax(sim)
    out = concat_heads(attn @ v) @ w_out + b_out

Sharding: 2 heads per core (tensor parallel). Each core:
  - computes LayerNorm(x) (replicated) and xn^T via PE transposes
  - projects its 2 heads' q/k/v (ln_scale folded into weights on host,
    ln_bias folded into a per-output-column bias added at PSUM evacuation)
  - attention in transposed layout (keys on partitions) so no transposes
    are needed between the two attention matmuls; a ones-column appended
    to v yields softmax denominators for free
  - AllGather of the per-head outputs (attn_out^T), chunked by row block
  - computes a 128-column slice of the final projection (+ bias)
Host assembles the 8 [128, 2048] outT shards into the [2048, 1024] output.
"""

import sys

import numpy as np

for _p in ("/opt/trn_rl_repo", "/root/.axon_site/_ro/trn_rl_repo"):
    if _p not in sys.path:
        sys.path.append(_p)

N = 2048          # sequence length
D = 1024          # model dim
HEADS = 16
DH = 64
NCORES = 8
HL = HEADS // NCORES          # heads per core (2)
HC = HL * DH                  # head cols per core (128)
LN_EPS = 1e-6
SIM_SCALE = float(D) ** -0.5  # reference scales by input dim

P = 128
RT = N // P        # 16 row tiles
DC = D // P        # 8 dim chunks
RC_W = 512         # row-chunk width for attention/collective pipeline
NRC = N // RC_W    # 4 row chunks

# Per-stage matmul compute dtype: float32r runs the PE at full rate for
# N>=256 (vs 4 cyc/row for exact float32) at reduced multiply precision.
# float32r inputs must be *stored* as float32r (producers round on write).
F32R_QKV = False
F32R_SIM = False
F32R_AV = False
F32R_PROJ = False

DEBUG_DUMPS = False

_BUILT = None


def _build():
    """Build the SPMD Bass graph (same graph on all 8 cores)."""
    from contextlib import ExitStack

    import concourse.bass as bass  # noqa: F401
    import concourse.tile as tile
    from concourse import bacc, mybir
    from concourse.masks import make_identity

    f32 = mybir.dt.float32
    f32r = mybir.dt.float32r
    dt_qkv = f32r if F32R_QKV else f32
    dt_sim = f32r if F32R_SIM else f32
    dt_av = f32r if F32R_AV else f32
    dt_proj = f32r if F32R_PROJ else f32
    AF = mybir.ActivationFunctionType

    nc = bacc.Bacc(None, num_devices=NCORES)

    x_d = nc.declare_dram_parameter("x", [N, D], f32, isOutput=False)
    wq_d = nc.declare_dram_parameter("wq", [D, HC], f32, isOutput=False)
    wk_d = nc.declare_dram_parameter("wk", [D, HC], f32, isOutput=False)
    wv_d = nc.declare_dram_parameter("wv", [D, HC], f32, isOutput=False)
    qb_d = nc.declare_dram_parameter("qb", [HC], f32, isOutput=False)
    kb_d = nc.declare_dram_parameter("kb", [HC], f32, isOutput=False)
    vb_d = nc.declare_dram_parameter("vb", [HC], f32, isOutput=False)
    wo_d = nc.declare_dram_parameter("wo", [D, HC], f32, isOutput=False)
    bo_d = nc.declare_dram_parameter("bo", [HC], f32, isOutput=False)
    out_d = nc.declare_dram_parameter("out", [HC, N], f32, isOutput=True)
    dbg = {}
    if DEBUG_DUMPS:
        for nm, shape in (("dbg_xnT0", [P, N]), ("dbg_qT", [P, N]),
                          ("dbg_kT", [P, N]), ("dbg_vT", [P, N]),
                          ("dbg_attn", [P, N]), ("dbg_ag0", [NCORES * HC, RC_W])):
            dbg[nm] = nc.declare_dram_parameter(nm, shape, f32, isOutput=True)

    groups = [list(range(NCORES))]

    with ExitStack() as ctx:
        tc = ctx.enter_context(tile.TileContext(nc))

        dram = ctx.enter_context(tc.tile_pool(name="dram", bufs=1, space="DRAM"))
        ag_in = [dram.tile([HC, RC_W], f32, name=f"ag_in{rc}") for rc in range(NRC)]
        ag_out = [
            dram.tile([NCORES * HC, RC_W], f32, addr_space="Shared",
                      name=f"ag_out{rc}")
            for rc in range(NRC)
        ]

        singles = ctx.enter_context(tc.tile_pool(name="singles", bufs=1))

        ident = singles.tile([P, P], f32)
        make_identity(nc, ident)
        eps_t = singles.tile([P, 1], f32)
        nc.vector.memset(eps_t, LN_EPS)
        ones_t = singles.tile([P, DH], f32)
        nc.vector.memset(ones_t, 1.0)

        # weights / biases
        wq_sb = singles.tile([P, DC, HC], dt_qkv)
        wk_sb = singles.tile([P, DC, HC], dt_qkv)
        wv_sb = singles.tile([P, DC, HC], dt_qkv)
        wo_sb = singles.tile([P, DC, HC], dt_proj)
        for w_sb, w_d in ((wq_sb, wq_d), (wk_sb, wk_d), (wv_sb, wv_d),
                          (wo_sb, wo_d)):
            nc.sync.dma_start(
                out=w_sb,
                in_=w_d[:, :].bitcast(w_sb.dtype).rearrange(
                    "(c p) m -> p c m", p=P
                ),
            )
        qb_t = singles.tile([P, 1], f32)
        kb_t = singles.tile([P, 1], f32)
        vb_t = singles.tile([P, 1], f32)
        bo_t = singles.tile([P, 1], f32)
        for b_t, b_d in ((qb_t, qb_d), (kb_t, kb_d), (vb_t, vb_d), (bo_t, bo_d)):
            nc.sync.dma_start(out=b_t, in_=b_d[:].rearrange("(p o) -> p o", o=1))

        # long-lived activations
        qT = singles.tile([P, N], dt_sim)       # [2*64 qdims, rows]
        kT = singles.tile([P, N], dt_sim)
        v_sb = singles.tile([P, RT, HL, DH + 1], dt_av)  # [rowchunk, rt, h, v|1]
        attn_h = [singles.tile([DH, N], f32, name=f"attn_h{h}") for h in range(HL)]
        outT = singles.tile([P, N], f32)

        nc.gpsimd.memset(v_sb[:, :, :, DH:].bitcast(f32), 1.0)  # ones column

        # ---- stages A-C: LayerNorm -> xn^T -> q/k/v projections ----
        with (
            tc.tile_pool(name="xp", bufs=3) as xp,
            tc.tile_pool(name="stat", bufs=4) as statp,
            tc.tile_pool(name="tp", bufs=2, space="PSUM") as tp,
            tc.tile_pool(name="mmp", bufs=2, space="PSUM") as mmp,
            tc.tile_pool(name="xnTp", bufs=1) as xnTp,
        ):
            xnT = xnTp.tile([P, DC, N], dt_qkv)   # [dim%128, dimchunk, rows]
            xnT_dbg = xnT

            for rt in range(RT):
                x_t = xp.tile([P, D], f32)
                nc.sync.dma_start(out=x_t, in_=x_d[rt * P:(rt + 1) * P, :])
                stats = statp.tile([P, 2, 6], f32)
                for sg in range(2):
                    nc.vector.bn_stats(
                        out=stats[:, sg, :], in_=x_t[:, sg * 512:(sg + 1) * 512]
                    )
                mv = statp.tile([P, 2], f32)
                nc.vector.bn_aggr(out=mv, in_=stats)
                rstd = statp.tile([P, 1], f32)
                nc.scalar.activation(
                    out=rstd, in_=mv[:, 1:2], func=AF.Sqrt, bias=eps_t, scale=1.0
                )
                nc.vector.reciprocal(out=rstd, in_=rstd)
                nc.vector.tensor_scalar(
                    out=x_t, in0=x_t,
                    scalar1=mv[:, 0:1], scalar2=rstd,
                    op0=mybir.AluOpType.subtract, op1=mybir.AluOpType.mult,
                )
                # transpose this row tile into xnT (8 [128,128] PE transposes,
                # batched 4 per PSUM bank; evacuation alternates ACT/DVE)
                for g in range(2):
                    pt = tp.tile([P, 512], f32, tag="pt")
                    for j in range(4):
                        dc = g * 4 + j
                        nc.tensor.transpose(
                            pt[:, j * P:(j + 1) * P],
                            x_t[:, dc * P:(dc + 1) * P],
                            ident,
                        )
                    dst = xnT[:, g * 4:(g + 1) * 4, rt * P:(rt + 1) * P]
                    src = pt[:].rearrange("p (j q) -> p j q", j=4)
                    if (rt + g) % 2 == 0:
                        nc.vector.tensor_copy(out=dst, in_=src)
                    else:
                        nc.scalar.copy(out=dst, in_=src)

            # q/k/v projections: out [128 cols, rows]
            vT = xnTp.tile([P, N], f32)
            vT_dbg = vT
            for w_sb, b_t, dst in (
                (wq_sb, qb_t, qT), (wk_sb, kb_t, kT), (wv_sb, vb_t, vT)
            ):
                for nt in range(N // 512):
                    pm = mmp.tile([P, 512], f32)
                    for kc in range(DC):
                        nc.tensor.matmul(
                            pm,
                            w_sb[:, kc, :],
                            xnT[:, kc, nt * 512:(nt + 1) * 512],
                            start=(kc == 0), stop=(kc == DC - 1),
                        )
                    nc.scalar.activation(
                        out=dst[:, nt * 512:(nt + 1) * 512], in_=pm,
                        func=AF.Identity, bias=b_t, scale=1.0,
                    )

            # v^T -> v (row-major with ones column): 16 PE transposes
            for rt in range(RT):
                pt = tp.tile([P, 512], f32, tag="pt")
                nc.tensor.transpose(
                    pt[:, :P], vT[:, rt * P:(rt + 1) * P], ident
                )
                nc.vector.tensor_copy(
                    out=v_sb[:, rt, :, 0:DH],
                    in_=pt[:, :P].rearrange("p (h d) -> p h d", h=HL),
                )

            if DEBUG_DUMPS:
                nc.sync.dma_start(out=dbg["dbg_xnT0"][:, :],
                                  in_=xnT[:, 0, :].bitcast(f32))
                nc.sync.dma_start(out=dbg["dbg_vT"][:, :], in_=vT[:].bitcast(f32))

        # ---- stage D: attention per (row chunk, head) + AG; stage F: proj ----
        with (
            tc.tile_pool(name="expp", bufs=2) as expp,
            tc.tile_pool(name="rsum", bufs=4) as rsump,
            tc.tile_pool(name="sp", bufs=3, space="PSUM") as sp,
            tc.tile_pool(name="op", bufs=2, space="PSUM") as op,
            tc.tile_pool(name="rp", bufs=1, space="PSUM") as rp,
            tc.tile_pool(name="agp", bufs=2) as agp,
            tc.tile_pool(name="fp", bufs=2, space="PSUM") as fp,
        ):
            for rc in range(NRC):
                r0 = rc * RC_W
                for h in range(HL):
                    hp = h * DH
                    exp_t = expp.tile([P, RT, RC_W], dt_av, tag="exp")
                    for kc in range(RT):
                        ps = sp.tile([P, RC_W], f32)
                        nc.tensor.matmul(
                            ps,
                            kT[hp:hp + DH, kc * P:(kc + 1) * P],
                            qT[hp:hp + DH, r0:r0 + RC_W],
                            start=True, stop=True,
                        )
                        nc.scalar.activation(
                            out=exp_t[:, kc, :], in_=ps,
                            func=AF.Exp, scale=SIM_SCALE,
                        )
                    po = op.tile([P, RC_W], f32)
                    for kc in range(RT):
                        nc.tensor.matmul(
                            po[0:DH + 1, :],
                            v_sb[:, kc, h, :],
                            exp_t[:, kc, :],
                            start=(kc == 0), stop=(kc == RT - 1),
                        )
                    # softmax denominators arrived in partition row DH of po;
                    # reciprocal, then partition-broadcast to DH rows via DMA
                    rs = rsump.tile([P, RC_W], f32, tag="rs")
                    nc.vector.reciprocal(out=rs[DH:DH + 1, :], in_=po[DH:DH + 1, :])
                    pr = rp.tile([DH, RC_W], f32, tag="pr")
                    nc.tensor.matmul(
                        pr, ones_t[DH:DH + 1, :], rs[DH:DH + 1, :],
                        start=True, stop=True,
                    )
                    rb = rsump.tile([DH, RC_W], f32, tag="rb")
                    nc.vector.tensor_copy(out=rb, in_=pr)
                    nc.vector.tensor_mul(
                        out=attn_h[h][:, r0:r0 + RC_W],
                        in0=po[0:DH, :], in1=rb,
                    )
                    nc.sync.dma_start(
                        out=ag_in[rc][hp:hp + DH, :],
                        in_=attn_h[h][:, r0:r0 + RC_W],
                    )
                nc.gpsimd.collective_compute(
                    "AllGather",
                    mybir.AluOpType.bypass,
                    replica_groups=groups,
                    ins=[ag_in[rc][:].opt()],
                    outs=[ag_out[rc][:].opt()],
                )
                # stage F for this row chunk: outT slice [128 cols, RC_W rows]
                agt = agp.tile([P, DC, RC_W], dt_proj, tag="agt")
                for kc in range(DC):
                    nc.sync.dma_start(
                        out=agt[:, kc, :],
                        in_=ag_out[rc][kc * P:(kc + 1) * P, :].bitcast(agt.dtype),
                    )
                pf = fp.tile([P, RC_W], f32)
                for kc in range(DC):
                    nc.tensor.matmul(
                        pf,
                        wo_sb[:, kc, :],
                        agt[:, kc, :],
                        start=(kc == 0), stop=(kc == DC - 1),
                    )
                nc.scalar.activation(
                    out=outT[:, r0:r0 + RC_W], in_=pf,
                    func=AF.Identity, bias=bo_t, scale=1.0,
                )
                nc.sync.dma_start(
                    out=out_d[:, r0:r0 + RC_W], in_=outT[:, r0:r0 + RC_W]
                )

            if DEBUG_DUMPS:
                nc.sync.dma_start(out=dbg["dbg_qT"][:, :], in_=qT[:].bitcast(f32))
                nc.sync.dma_start(out=dbg["dbg_kT"][:, :], in_=kT[:].bitcast(f32))
                for h in range(HL):
                    nc.sync.dma_start(out=dbg["dbg_attn"][h * DH:(h + 1) * DH, :],
                                      in_=attn_h[h][:])
                nc.sync.dma_start(out=dbg["dbg_ag0"][:, :], in_=ag_out[0][:])

    if not nc.is_finalized():
        nc.finalize()
    return nc


def _get_built():
    global _BUILT
    if _BUILT is None:
        _BUILT = _build()
    return _BUILT


def _shard_inputs(x, ln_scale, ln_bias, w_qkv, w_out, b_out):
    """Host-side sharding: slice per-head weight columns, fold LN params."""
    x = np.ascontiguousarray(np.asarray(x, np.float32))
    ln_scale = np.asarray(ln_scale, np.float32)
    ln_bias = np.asarray(ln_bias, np.float32)
    w_qkv = np.asarray(w_qkv, np.float32)
    w_out = np.asarray(w_out, np.float32)
    b_out = np.asarray(b_out, np.float32)

    in_maps = []
    for ci in range(NCORES):
        c0 = ci * HC
        sl = {}
        for name, off in (("q", 0), ("k", HEADS * DH), ("v", 2 * HEADS * DH)):
            w = w_qkv[:, off + c0: off + c0 + HC]
            sl["w" + name] = np.ascontiguousarray(ln_scale[:, None] * w)
            sl[name + "b"] = np.ascontiguousarray(ln_bias @ w)
        sl["wo"] = np.ascontiguousarray(w_out[:, c0:c0 + HC])
        sl["bo"] = np.ascontiguousarray(b_out[c0:c0 + HC])
        sl["x"] = x
        in_maps.append(sl)
    return in_maps


def kernel(x, ln_scale, ln_bias, w_qkv, w_out, b_out):
    from concourse.bass_utils import run_bass_kernel_spmd

    nc = _get_built()
    in_maps = _shard_inputs(x, ln_scale, ln_bias, w_qkv, w_out, b_out)
    res = run_bass_kernel_spmd(nc, in_maps, core_ids=list(range(NCORES)))
    shards = [res.results[ci]["out"] for ci in range(NCORES)]  # [128, 2048] each
    outT = np.concatenate(shards, axis=0)  # [1024, 2048]
    return np.ascontiguousarray(outT.T)


# revision 15
# speedup vs baseline: 1.7406x; 1.7406x over previous
"""Distributed Trainium2 kernel for pre-LN multi-head self-attention.

Reference computation (n=2048, d=1024, 16 heads x 64):
    xn  = LayerNorm(x) * ln_scale + ln_bias
    qkv = xn @ w_qkv ; split -> q,k,v [16, 2048, 64]
    sim = (q @ k^T) * d**-0.5 ; attn = softmax(sim)
    out = concat_heads(attn @ v) @ w_out + b_out

Sharding: 2 heads per core (tensor parallel). Each core:
  - computes LayerNorm(x) (replicated) and xn^T via PE transposes
  - projects its 2 heads' q/k/v (ln_scale folded into weights on host,
    ln_bias folded into a per-output-column bias added at PSUM evacuation)
  - attention in transposed layout (keys on partitions) so no transposes
    are needed between the two attention matmuls; a ones-column appended
    to v yields softmax denominators for free
  - AllGather of the per-head outputs (attn_out^T), chunked by row block
  - computes a 128-column slice of the final projection (+ bias)
Host assembles the 8 [128, 2048] outT shards into the [2048, 1024] output.
"""

import sys

import numpy as np

for _p in ("/opt/trn_rl_repo", "/root/.axon_site/_ro/trn_rl_repo"):
    if _p not in sys.path:
        sys.path.append(_p)

N = 2048          # sequence length
D = 1024          # model dim
HEADS = 16
DH = 64
NCORES = 8
HL = HEADS // NCORES          # heads per core (2)
HC = HL * DH                  # head cols per core (128)
LN_EPS = 1e-6
SIM_SCALE = float(D) ** -0.5  # reference scales by input dim

P = 128
RT = N // P        # 16 row tiles
DC = D // P        # 8 dim chunks
RC_W = 512         # row-chunk width for attention/collective pipeline
NRC = N // RC_W    # 4 row chunks

# Per-stage matmul compute dtype: float32r runs the PE at full rate for
# N>=256 (vs 4 cyc/row for exact float32) at reduced multiply precision.
# float32r inputs must be *stored* as float32r (producers round on write).
F32R_QKV = True
F32R_SIM = True
F32R_AV = True
F32R_PROJ = True

DEBUG_DUMPS = False

_BUILT = None


def _build():
    """Build the SPMD Bass graph (same graph on all 8 cores)."""
    from contextlib import ExitStack

    import concourse.bass as bass  # noqa: F401
    import concourse.tile as tile
    from concourse import bacc, mybir
    from concourse.masks import make_identity

    f32 = mybir.dt.float32
    f32r = mybir.dt.float32r
    dt_qkv = f32r if F32R_QKV else f32
    dt_sim = f32r if F32R_SIM else f32
    dt_av = f32r if F32R_AV else f32
    dt_proj = f32r if F32R_PROJ else f32
    AF = mybir.ActivationFunctionType

    nc = bacc.Bacc(None, num_devices=NCORES)

    x_d = nc.declare_dram_parameter("x", [N, D], f32, isOutput=False)
    wq_d = nc.declare_dram_parameter("wq", [D, HC], f32, isOutput=False)
    wk_d = nc.declare_dram_parameter("wk", [D, HC], f32, isOutput=False)
    wv_d = nc.declare_dram_parameter("wv", [D, HC], f32, isOutput=False)
    qb_d = nc.declare_dram_parameter("qb", [HC], f32, isOutput=False)
    kb_d = nc.declare_dram_parameter("kb", [HC], f32, isOutput=False)
    vb_d = nc.declare_dram_parameter("vb", [HC], f32, isOutput=False)
    wo_d = nc.declare_dram_parameter("wo", [D, HC], f32, isOutput=False)
    bo_d = nc.declare_dram_parameter("bo", [HC], f32, isOutput=False)
    out_d = nc.declare_dram_parameter("out", [HC, N], f32, isOutput=True)
    dbg = {}
    if DEBUG_DUMPS:
        for nm, shape in (("dbg_xnT0", [P, N]), ("dbg_qT", [P, N]),
                          ("dbg_kT", [P, N]), ("dbg_vT", [P, N]),
                          ("dbg_attn", [P, N]), ("dbg_ag0", [NCORES * HC, RC_W])):
            dbg[nm] = nc.declare_dram_parameter(nm, shape, f32, isOutput=True)

    groups = [list(range(NCORES))]

    with ExitStack() as ctx:
        tc = ctx.enter_context(tile.TileContext(nc))

        dram = ctx.enter_context(tc.tile_pool(name="dram", bufs=1, space="DRAM"))
        ag_in = [dram.tile([HC, RC_W], f32, name=f"ag_in{rc}") for rc in range(NRC)]
        ag_out = [
            dram.tile([NCORES * HC, RC_W], f32, addr_space="Shared",
                      name=f"ag_out{rc}")
            for rc in range(NRC)
        ]

        singles = ctx.enter_context(tc.tile_pool(name="singles", bufs=1))

        ident = singles.tile([P, P], f32)
        make_identity(nc, ident)
        eps_t = singles.tile([P, 1], f32)
        nc.vector.memset(eps_t, LN_EPS)
        ones_t = singles.tile([P, DH], f32)
        nc.vector.memset(ones_t, 1.0)

        # weights / biases
        wq_sb = singles.tile([P, DC, HC], dt_qkv)
        wk_sb = singles.tile([P, DC, HC], dt_qkv)
        wv_sb = singles.tile([P, DC, HC], dt_qkv)
        wo_sb = singles.tile([P, DC, HC], dt_proj)
        for w_sb, w_d in ((wq_sb, wq_d), (wk_sb, wk_d), (wv_sb, wv_d),
                          (wo_sb, wo_d)):
            nc.sync.dma_start(
                out=w_sb,
                in_=w_d[:, :].bitcast(w_sb.dtype).rearrange(
                    "(c p) m -> p c m", p=P
                ),
            )
        qb_t = singles.tile([P, 1], f32)
        kb_t = singles.tile([P, 1], f32)
        vb_t = singles.tile([P, 1], f32)
        bo_t = singles.tile([P, 1], f32)
        for b_t, b_d in ((qb_t, qb_d), (kb_t, kb_d), (vb_t, vb_d), (bo_t, bo_d)):
            nc.sync.dma_start(out=b_t, in_=b_d[:].rearrange("(p o) -> p o", o=1))

        # long-lived activations
        qT = singles.tile([P, N], dt_sim)       # [2*64 qdims, rows]
        kT = singles.tile([P, N], dt_sim)
        v_sb = singles.tile([P, RT, HL, DH + 1], dt_av)  # [rowchunk, rt, h, v|1]
        attn_h = [singles.tile([DH, N], f32, name=f"attn_h{h}") for h in range(HL)]
        outT = singles.tile([P, N], f32)

        nc.gpsimd.memset(v_sb[:, :, :, DH:].bitcast(f32), 1.0)  # ones column

        # ---- stages A-C: LayerNorm -> xn^T -> q/k/v projections ----
        with (
            tc.tile_pool(name="xp", bufs=3) as xp,
            tc.tile_pool(name="stat", bufs=4) as statp,
            tc.tile_pool(name="tp", bufs=2, space="PSUM") as tp,
            tc.tile_pool(name="mmp", bufs=2, space="PSUM") as mmp,
            tc.tile_pool(name="xnTp", bufs=1) as xnTp,
        ):
            xnT = xnTp.tile([P, DC, N], dt_qkv)   # [dim%128, dimchunk, rows]
            xnT_dbg = xnT

            for rt in range(RT):
                x_t = xp.tile([P, D], f32)
                nc.sync.dma_start(out=x_t, in_=x_d[rt * P:(rt + 1) * P, :])
                stats = statp.tile([P, 2, 6], f32)
                for sg in range(2):
                    nc.vector.bn_stats(
                        out=stats[:, sg, :], in_=x_t[:, sg * 512:(sg + 1) * 512]
                    )
                mv = statp.tile([P, 2], f32)
                nc.vector.bn_aggr(out=mv, in_=stats)
                rstd = statp.tile([P, 1], f32)
                nc.scalar.activation(
                    out=rstd, in_=mv[:, 1:2], func=AF.Sqrt, bias=eps_t, scale=1.0
                )
                nc.vector.reciprocal(out=rstd, in_=rstd)
                nc.vector.tensor_scalar(
                    out=x_t, in0=x_t,
                    scalar1=mv[:, 0:1], scalar2=rstd,
                    op0=mybir.AluOpType.subtract, op1=mybir.AluOpType.mult,
                )
                # transpose this row tile into xnT (8 [128,128] PE transposes,
                # batched 4 per PSUM bank; evacuation alternates ACT/DVE)
                for g in range(2):
                    pt = tp.tile([P, 512], f32, tag="pt")
                    for j in range(4):
                        dc = g * 4 + j
                        nc.tensor.transpose(
                            pt[:, j * P:(j + 1) * P],
                            x_t[:, dc * P:(dc + 1) * P],
                            ident,
                        )
                    dst = xnT[:, g * 4:(g + 1) * 4, rt * P:(rt + 1) * P]
                    src = pt[:].rearrange("p (j q) -> p j q", j=4)
                    if (rt + g) % 2 == 0:
                        nc.vector.tensor_copy(out=dst, in_=src)
                    else:
                        nc.scalar.copy(out=dst, in_=src)

            # q/k/v projections: out [128 cols, rows]
            vT = xnTp.tile([P, N], f32)
            vT_dbg = vT
            for w_sb, b_t, dst in (
                (wq_sb, qb_t, qT), (wk_sb, kb_t, kT), (wv_sb, vb_t, vT)
            ):
                for nt in range(N // 512):
                    pm = mmp.tile([P, 512], f32)
                    for kc in range(DC):
                        nc.tensor.matmul(
                            pm,
                            w_sb[:, kc, :],
                            xnT[:, kc, nt * 512:(nt + 1) * 512],
                            start=(kc == 0), stop=(kc == DC - 1),
                        )
                    nc.scalar.activation(
                        out=dst[:, nt * 512:(nt + 1) * 512], in_=pm,
                        func=AF.Identity, bias=b_t, scale=1.0,
                    )

            # v^T -> v (row-major with ones column): 16 PE transposes
            for rt in range(RT):
                pt = tp.tile([P, 512], f32, tag="pt")
                nc.tensor.transpose(
                    pt[:, :P], vT[:, rt * P:(rt + 1) * P], ident
                )
                nc.vector.tensor_copy(
                    out=v_sb[:, rt, :, 0:DH],
                    in_=pt[:, :P].rearrange("p (h d) -> p h d", h=HL),
                )

            if DEBUG_DUMPS:
                nc.sync.dma_start(out=dbg["dbg_xnT0"][:, :],
                                  in_=xnT[:, 0, :].bitcast(f32))
                nc.sync.dma_start(out=dbg["dbg_vT"][:, :], in_=vT[:].bitcast(f32))

        # ---- stage D: attention per (row chunk, head) + AG; stage F: proj ----
        with (
            tc.tile_pool(name="expp", bufs=2) as expp,
            tc.tile_pool(name="rsum", bufs=4) as rsump,
            tc.tile_pool(name="sp", bufs=3, space="PSUM") as sp,
            tc.tile_pool(name="op", bufs=2, space="PSUM") as op,
            tc.tile_pool(name="rp", bufs=1, space="PSUM") as rp,
            tc.tile_pool(name="agp", bufs=2) as agp,
            tc.tile_pool(name="fp", bufs=2, space="PSUM") as fp,
        ):
            for rc in range(NRC):
                r0 = rc * RC_W
                for h in range(HL):
                    hp = h * DH
                    exp_t = expp.tile([P, RT, RC_W], dt_av, tag="exp")
                    for kc in range(RT):
                        ps = sp.tile([P, RC_W], f32)
                        nc.tensor.matmul(
                            ps,
                            kT[hp:hp + DH, kc * P:(kc + 1) * P],
                            qT[hp:hp + DH, r0:r0 + RC_W],
                            start=True, stop=True,
                        )
                        nc.scalar.activation(
                            out=exp_t[:, kc, :], in_=ps,
                            func=AF.Exp, scale=SIM_SCALE,
                        )
                    po = op.tile([P, RC_W], f32)
                    for kc in range(RT):
                        nc.tensor.matmul(
                            po[0:DH + 1, :],
                            v_sb[:, kc, h, :],
                            exp_t[:, kc, :],
                            start=(kc == 0), stop=(kc == RT - 1),
                        )
                    # softmax denominators arrived in partition row DH of po;
                    # reciprocal, then partition-broadcast to DH rows via DMA
                    rs = rsump.tile([P, RC_W], f32, tag="rs")
                    nc.vector.reciprocal(out=rs[DH:DH + 1, :], in_=po[DH:DH + 1, :])
                    pr = rp.tile([DH, RC_W], f32, tag="pr")
                    nc.tensor.matmul(
                        pr, ones_t[DH:DH + 1, :], rs[DH:DH + 1, :],
                        start=True, stop=True,
                    )
                    rb = rsump.tile([DH, RC_W], f32, tag="rb")
                    nc.vector.tensor_copy(out=rb, in_=pr)
                    nc.vector.tensor_mul(
                        out=attn_h[h][:, r0:r0 + RC_W],
                        in0=po[0:DH, :], in1=rb,
                    )
                    nc.sync.dma_start(
                        out=ag_in[rc][hp:hp + DH, :],
                        in_=attn_h[h][:, r0:r0 + RC_W],
                    )
                nc.gpsimd.collective_compute(
                    "AllGather",
                    mybir.AluOpType.bypass,
                    replica_groups=groups,
                    ins=[ag_in[rc][:].opt()],
                    outs=[ag_out[rc][:].opt()],
                )
                # stage F for this row chunk: outT slice [128 cols, RC_W rows]
                agt = agp.tile([P, DC, RC_W], dt_proj, tag="agt")
                for kc in range(DC):
                    nc.sync.dma_start(
                        out=agt[:, kc, :],
                        in_=ag_out[rc][kc * P:(kc + 1) * P, :].bitcast(agt.dtype),
                    )
                pf = fp.tile([P, RC_W], f32)
                for kc in range(DC):
                    nc.tensor.matmul(
                        pf,
                        wo_sb[:, kc, :],
                        agt[:, kc, :],
                        start=(kc == 0), stop=(kc == DC - 1),
                    )
                nc.scalar.activation(
                    out=outT[:, r0:r0 + RC_W], in_=pf,
                    func=AF.Identity, bias=bo_t, scale=1.0,
                )
                nc.sync.dma_start(
                    out=out_d[:, r0:r0 + RC_W], in_=outT[:, r0:r0 + RC_W]
                )

            if DEBUG_DUMPS:
                nc.sync.dma_start(out=dbg["dbg_qT"][:, :], in_=qT[:].bitcast(f32))
                nc.sync.dma_start(out=dbg["dbg_kT"][:, :], in_=kT[:].bitcast(f32))
                for h in range(HL):
                    nc.sync.dma_start(out=dbg["dbg_attn"][h * DH:(h + 1) * DH, :],
                                      in_=attn_h[h][:])
                nc.sync.dma_start(out=dbg["dbg_ag0"][:, :], in_=ag_out[0][:])

    if not nc.is_finalized():
        nc.finalize()
    return nc


def _get_built():
    global _BUILT
    if _BUILT is None:
        _BUILT = _build()
    return _BUILT


def _shard_inputs(x, ln_scale, ln_bias, w_qkv, w_out, b_out):
    """Host-side sharding: slice per-head weight columns, fold LN params."""
    x = np.ascontiguousarray(np.asarray(x, np.float32))
    ln_scale = np.asarray(ln_scale, np.float32)
    ln_bias = np.asarray(ln_bias, np.float32)
    w_qkv = np.asarray(w_qkv, np.float32)
    w_out = np.asarray(w_out, np.float32)
    b_out = np.asarray(b_out, np.float32)

    in_maps = []
    for ci in range(NCORES):
        c0 = ci * HC
        sl = {}
        for name, off in (("q", 0), ("k", HEADS * DH), ("v", 2 * HEADS * DH)):
            w = w_qkv[:, off + c0: off + c0 + HC]
            sl["w" + name] = np.ascontiguousarray(ln_scale[:, None] * w)
            sl[name + "b"] = np.ascontiguousarray(ln_bias @ w)
        sl["wo"] = np.ascontiguousarray(w_out[:, c0:c0 + HC])
        sl["bo"] = np.ascontiguousarray(b_out[c0:c0 + HC])
        sl["x"] = x
        in_maps.append(sl)
    return in_maps


def kernel(x, ln_scale, ln_bias, w_qkv, w_out, b_out):
    from concourse.bass_utils import run_bass_kernel_spmd

    nc = _get_built()
    in_maps = _shard_inputs(x, ln_scale, ln_bias, w_qkv, w_out, b_out)
    res = run_bass_kernel_spmd(nc, in_maps, core_ids=list(range(NCORES)))
    shards = [res.results[ci]["out"] for ci in range(NCORES)]  # [128, 2048] each
    outT = np.concatenate(shards, axis=0)  # [1024, 2048]
    return np.ascontiguousarray(outT.T)


# revision 20
# speedup vs baseline: 1.9786x; 1.1367x over previous
"""Distributed Trainium2 kernel for pre-LN multi-head self-attention.

Reference computation (n=2048, d=1024, 16 heads x 64):
    xn  = LayerNorm(x) * ln_scale + ln_bias
    qkv = xn @ w_qkv ; split -> q,k,v [16, 2048, 64]
    sim = (q @ k^T) * d**-0.5 ; attn = softmax(sim)
    out = concat_heads(attn @ v) @ w_out + b_out

Sharding: 2 heads per core (tensor parallel). Each core:
  - computes LayerNorm(x) (replicated) and xn^T via PE transposes
  - projects its 2 heads' q/k/v (ln_scale folded into weights on host,
    ln_bias folded into a per-output-column bias added at PSUM evacuation)
  - attention in transposed layout (keys on partitions) so no transposes
    are needed between the two attention matmuls; a ones-column appended
    to v yields softmax denominators for free
  - AllGather of the per-head outputs (attn_out^T), chunked by row block
  - computes a 128-column slice of the final projection (+ bias)
Host assembles the 8 [128, 2048] outT shards into the [2048, 1024] output.
"""

import sys

import ml_dtypes
import numpy as np

for _p in ("/opt/trn_rl_repo", "/root/.axon_site/_ro/trn_rl_repo"):
    if _p not in sys.path:
        sys.path.append(_p)

N = 2048          # sequence length
D = 1024          # model dim
HEADS = 16
DH = 64
NCORES = 8
HL = HEADS // NCORES          # heads per core (2)
HC = HL * DH                  # head cols per core (128)
LN_EPS = 1e-6
SIM_SCALE = float(D) ** -0.5  # reference scales by input dim

P = 128
RT = N // P        # 16 row tiles
DC = D // P        # 8 dim chunks
RC_W = 512         # row-chunk width for attention/collective pipeline
NRC = N // RC_W    # 4 row chunks

# Per-stage matmul compute dtype: "bf16" runs the PE in standard mode at
# full rate (and keeps the HAM clock warm - fp32 modes do not), "f32r" is
# single-pass reduced-precision fp32, "f32" is exact 4-pass fp32.
MM_QKV = "bf16"
MM_SIM = "bf16"
MM_AV = "bf16"
MM_PROJ = "bf16"

DEBUG_DUMPS = False

_BUILT = None


def _build():
    """Build the SPMD Bass graph (same graph on all 8 cores)."""
    from contextlib import ExitStack

    import concourse.bass as bass  # noqa: F401
    import concourse.tile as tile
    from concourse import bacc, mybir
    from concourse.masks import make_identity

    f32 = mybir.dt.float32
    DT = {"f32": f32, "f32r": mybir.dt.float32r, "bf16": mybir.dt.bfloat16}
    dt_qkv = DT[MM_QKV]
    dt_sim = DT[MM_SIM]
    dt_av = DT[MM_AV]
    dt_proj = DT[MM_PROJ]
    AF = mybir.ActivationFunctionType

    nc = bacc.Bacc(None, num_devices=NCORES)

    x_d = nc.declare_dram_parameter("x", [N, D], f32, isOutput=False)
    wq_d = nc.declare_dram_parameter("wq", [D, HC], dt_qkv, isOutput=False)
    wk_d = nc.declare_dram_parameter("wk", [D, HC], dt_qkv, isOutput=False)
    wv_d = nc.declare_dram_parameter("wv", [D, HC], dt_qkv, isOutput=False)
    qb_d = nc.declare_dram_parameter("qb", [HC], f32, isOutput=False)
    kb_d = nc.declare_dram_parameter("kb", [HC], f32, isOutput=False)
    vb_d = nc.declare_dram_parameter("vb", [HC], f32, isOutput=False)
    wo_d = nc.declare_dram_parameter("wo", [D, HC], dt_proj, isOutput=False)
    bo_d = nc.declare_dram_parameter("bo", [HC], f32, isOutput=False)
    out_d = nc.declare_dram_parameter("out", [HC, N], f32, isOutput=True)
    dbg = {}
    if DEBUG_DUMPS:
        for nm, shape in (("dbg_xnT0", [P, N]), ("dbg_qT", [P, N]),
                          ("dbg_kT", [P, N]), ("dbg_vT", [P, N]),
                          ("dbg_attn", [P, N]), ("dbg_ag0", [NCORES * HC, RC_W])):
            dbg[nm] = nc.declare_dram_parameter(nm, shape, f32, isOutput=True)

    groups = [list(range(NCORES))]

    with ExitStack() as ctx:
        tc = ctx.enter_context(tile.TileContext(nc))

        dram = ctx.enter_context(tc.tile_pool(name="dram", bufs=1, space="DRAM"))
        ag_in = [dram.tile([HC, RC_W], dt_av, name=f"ag_in{rc}")
                 for rc in range(NRC)]
        ag_out = [
            dram.tile([NCORES * HC, RC_W], dt_av, addr_space="Shared",
                      name=f"ag_out{rc}")
            for rc in range(NRC)
        ]

        singles = ctx.enter_context(tc.tile_pool(name="singles", bufs=1))

        ident = singles.tile([P, P], dt_qkv)
        make_identity(nc, ident)
        eps_t = singles.tile([P, 1], f32)
        nc.vector.memset(eps_t, LN_EPS)
        ones_t = singles.tile([P, DH], dt_av)
        nc.vector.memset(ones_t, 1.0)
        ident_av = ident
        if dt_av != dt_qkv:
            ident_av = singles.tile([P, P], dt_av, name="ident_av")
            make_identity(nc, ident_av)

        # weights / biases
        wq_sb = singles.tile([P, DC, HC], dt_qkv)
        wk_sb = singles.tile([P, DC, HC], dt_qkv)
        wv_sb = singles.tile([P, DC, HC], dt_qkv)
        wo_sb = singles.tile([P, DC, HC], dt_proj)
        for w_sb, w_d in ((wq_sb, wq_d), (wk_sb, wk_d), (wv_sb, wv_d),
                          (wo_sb, wo_d)):
            in_ap = w_d[:, :]
            if in_ap.dtype != w_sb.dtype:
                in_ap = in_ap.bitcast(w_sb.dtype)
            nc.sync.dma_start(
                out=w_sb, in_=in_ap.rearrange("(c p) m -> p c m", p=P)
            )
        qb_t = singles.tile([P, 1], f32)
        kb_t = singles.tile([P, 1], f32)
        vb_t = singles.tile([P, 1], f32)
        bo_t = singles.tile([P, 1], f32)
        for b_t, b_d in ((qb_t, qb_d), (kb_t, kb_d), (vb_t, vb_d), (bo_t, bo_d)):
            nc.sync.dma_start(out=b_t, in_=b_d[:].rearrange("(p o) -> p o", o=1))

        # long-lived activations
        qT = singles.tile([P, N], dt_sim)       # [2*64 qdims, rows]
        kT = singles.tile([P, N], dt_sim)
        v_sb = singles.tile([P, RT, HL, DH + 1], dt_av)  # [rowchunk, rt, h, v|1]
        attn_h = [singles.tile([DH, N], dt_av, name=f"attn_h{h}")
                  for h in range(HL)]
        outT = singles.tile([P, N], f32)

        ones_col = v_sb[:, :, :, DH:]
        if ones_col.dtype == mybir.dt.float32r:
            ones_col = ones_col.bitcast(f32)
        nc.gpsimd.memset(ones_col, 1.0)  # ones column

        # ---- stages A-C: LayerNorm -> xn^T -> q/k/v projections ----
        with (
            tc.tile_pool(name="xp", bufs=3) as xp,
            tc.tile_pool(name="stat", bufs=4) as statp,
            tc.tile_pool(name="tp", bufs=2, space="PSUM") as tp,
            tc.tile_pool(name="mmp", bufs=2, space="PSUM") as mmp,
            tc.tile_pool(name="xnTp", bufs=1) as xnTp,
        ):
            xnT = xnTp.tile([P, DC, N], dt_qkv)   # [dim%128, dimchunk, rows]
            xnT_dbg = xnT

            for rt in range(RT):
                x_t = xp.tile([P, D], f32)
                nc.sync.dma_start(out=x_t, in_=x_d[rt * P:(rt + 1) * P, :])
                stats = statp.tile([P, 2, 6], f32)
                for sg in range(2):
                    nc.vector.bn_stats(
                        out=stats[:, sg, :], in_=x_t[:, sg * 512:(sg + 1) * 512]
                    )
                mv = statp.tile([P, 2], f32)
                nc.vector.bn_aggr(out=mv, in_=stats)
                rstd = statp.tile([P, 1], f32)
                nc.scalar.activation(
                    out=rstd, in_=mv[:, 1:2], func=AF.Sqrt, bias=eps_t, scale=1.0
                )
                nc.vector.reciprocal(out=rstd, in_=rstd)
                xh_t = xp.tile([P, D], dt_qkv, tag="xh")
                nc.vector.tensor_scalar(
                    out=xh_t, in0=x_t,
                    scalar1=mv[:, 0:1], scalar2=rstd,
                    op0=mybir.AluOpType.subtract, op1=mybir.AluOpType.mult,
                )
                # transpose this row tile into xnT (8 [128,128] PE transposes,
                # batched 4 per PSUM bank; evacuation alternates ACT/DVE)
                for g in range(2):
                    pt = tp.tile([P, 512], dt_qkv, tag="pt")
                    with nc.allow_low_precision(reason="transpose is a copy"):
                        for j in range(4):
                            dc = g * 4 + j
                            nc.tensor.transpose(
                                pt[:, j * P:(j + 1) * P],
                                xh_t[:, dc * P:(dc + 1) * P],
                                ident,
                            )
                    dst = xnT[:, g * 4:(g + 1) * 4, rt * P:(rt + 1) * P]
                    src = pt[:].rearrange("p (j q) -> p j q", j=4)
                    if (rt + g) % 2 == 0:
                        nc.vector.tensor_copy(out=dst, in_=src)
                    else:
                        nc.scalar.copy(out=dst, in_=src)

            # q/k/v projections: out [128 cols, rows]
            vT = xnTp.tile([P, N], dt_av)
            vT_dbg = vT
            for w_sb, b_t, dst in (
                (wq_sb, qb_t, qT), (wk_sb, kb_t, kT), (wv_sb, vb_t, vT)
            ):
                for nt in range(N // 512):
                    pm = mmp.tile([P, 512], f32)
                    for kc in range(DC):
                        nc.tensor.matmul(
                            pm,
                            w_sb[:, kc, :],
                            xnT[:, kc, nt * 512:(nt + 1) * 512],
                            start=(kc == 0), stop=(kc == DC - 1),
                        )
                    nc.scalar.activation(
                        out=dst[:, nt * 512:(nt + 1) * 512], in_=pm,
                        func=AF.Identity, bias=b_t, scale=1.0,
                    )

            # v^T -> v (row-major with ones column): 16 PE transposes
            for rt in range(RT):
                pt = tp.tile([P, 512], dt_av, tag="pt")
                with nc.allow_low_precision(reason="transpose is a copy"):
                    nc.tensor.transpose(
                        pt[:, :P], vT[:, rt * P:(rt + 1) * P],
                        ident if vT.dtype == ident.dtype else ident_av,
                    )
                nc.vector.tensor_copy(
                    out=v_sb[:, rt, :, 0:DH],
                    in_=pt[:, :P].rearrange("p (h d) -> p h d", h=HL),
                )

            if DEBUG_DUMPS:
                nc.sync.dma_start(out=dbg["dbg_xnT0"][:, :],
                                  in_=xnT[:, 0, :].bitcast(f32))
                nc.sync.dma_start(out=dbg["dbg_vT"][:, :], in_=vT[:].bitcast(f32))

        # ---- stage D: attention per (row chunk, head) + AG; stage F: proj ----
        with (
            tc.tile_pool(name="expp", bufs=2) as expp,
            tc.tile_pool(name="rsum", bufs=4) as rsump,
            tc.tile_pool(name="sp", bufs=3, space="PSUM") as sp,
            tc.tile_pool(name="op", bufs=2, space="PSUM") as op,
            tc.tile_pool(name="rp", bufs=1, space="PSUM") as rp,
            tc.tile_pool(name="agp", bufs=2) as agp,
            tc.tile_pool(name="fp", bufs=2, space="PSUM") as fp,
        ):
            for rc in range(NRC):
                r0 = rc * RC_W
                for h in range(HL):
                    hp = h * DH
                    exp_t = expp.tile([P, RT, RC_W], dt_av, tag="exp")
                    for kc in range(RT):
                        ps = sp.tile([P, RC_W], f32)
                        nc.tensor.matmul(
                            ps,
                            kT[hp:hp + DH, kc * P:(kc + 1) * P],
                            qT[hp:hp + DH, r0:r0 + RC_W],
                            start=True, stop=True,
                        )
                        nc.scalar.activation(
                            out=exp_t[:, kc, :], in_=ps,
                            func=AF.Exp, scale=SIM_SCALE,
                        )
                    po = op.tile([P, RC_W], f32)
                    for kc in range(RT):
                        nc.tensor.matmul(
                            po[0:DH + 1, :],
                            v_sb[:, kc, h, :],
                            exp_t[:, kc, :],
                            start=(kc == 0), stop=(kc == RT - 1),
                        )
                    # softmax denominators arrived in partition row DH of po;
                    # reciprocal, then partition-broadcast to DH rows via DMA
                    rs = rsump.tile([P, RC_W], dt_av, tag="rs")
                    with nc.allow_low_precision(reason="softmax recips"):
                        nc.vector.reciprocal(
                            out=rs[DH:DH + 1, :], in_=po[DH:DH + 1, :]
                        )
                    pr = rp.tile([DH, RC_W], f32, tag="pr")
                    nc.tensor.matmul(
                        pr, ones_t[DH:DH + 1, :], rs[DH:DH + 1, :],
                        start=True, stop=True,
                    )
                    rb = rsump.tile([DH, RC_W], f32, tag="rb")
                    nc.vector.tensor_copy(out=rb, in_=pr)
                    with nc.allow_low_precision(reason="attn bf16 wire"):
                        nc.vector.tensor_mul(
                            out=attn_h[h][:, r0:r0 + RC_W],
                            in0=po[0:DH, :], in1=rb,
                        )
                    nc.sync.dma_start(
                        out=ag_in[rc][hp:hp + DH, :],
                        in_=attn_h[h][:, r0:r0 + RC_W],
                    )
                nc.gpsimd.collective_compute(
                    "AllGather",
                    mybir.AluOpType.bypass,
                    replica_groups=groups,
                    ins=[ag_in[rc][:].opt()],
                    outs=[ag_out[rc][:].opt()],
                )
                # stage F for this row chunk: outT slice [128 cols, RC_W rows]
                agt = agp.tile([P, DC, RC_W], dt_proj, tag="agt")
                for kc in range(DC):
                    ag_src = ag_out[rc][kc * P:(kc + 1) * P, :]
                    if ag_src.dtype != agt.dtype:
                        ag_src = ag_src.bitcast(agt.dtype)
                    nc.sync.dma_start(out=agt[:, kc, :], in_=ag_src)
                pf = fp.tile([P, RC_W], f32)
                for kc in range(DC):
                    nc.tensor.matmul(
                        pf,
                        wo_sb[:, kc, :],
                        agt[:, kc, :],
                        start=(kc == 0), stop=(kc == DC - 1),
                    )
                nc.scalar.activation(
                    out=outT[:, r0:r0 + RC_W], in_=pf,
                    func=AF.Identity, bias=bo_t, scale=1.0,
                )
                nc.sync.dma_start(
                    out=out_d[:, r0:r0 + RC_W], in_=outT[:, r0:r0 + RC_W]
                )

            if DEBUG_DUMPS:
                nc.sync.dma_start(out=dbg["dbg_qT"][:, :], in_=qT[:].bitcast(f32))
                nc.sync.dma_start(out=dbg["dbg_kT"][:, :], in_=kT[:].bitcast(f32))
                for h in range(HL):
                    nc.sync.dma_start(out=dbg["dbg_attn"][h * DH:(h + 1) * DH, :],
                                      in_=attn_h[h][:])
                nc.sync.dma_start(out=dbg["dbg_ag0"][:, :], in_=ag_out[0][:])

    if not nc.is_finalized():
        nc.finalize()
    return nc


def _get_built():
    global _BUILT
    if _BUILT is None:
        _BUILT = _build()
    return _BUILT


def _shard_inputs(x, ln_scale, ln_bias, w_qkv, w_out, b_out):
    """Host-side sharding: slice per-head weight columns, fold LN params."""
    x = np.ascontiguousarray(np.asarray(x, np.float32))
    ln_scale = np.asarray(ln_scale, np.float32)
    ln_bias = np.asarray(ln_bias, np.float32)
    w_qkv = np.asarray(w_qkv, np.float32)
    w_out = np.asarray(w_out, np.float32)
    b_out = np.asarray(b_out, np.float32)

    w_dt = {"f32": np.float32, "f32r": np.float32,
            "bf16": ml_dtypes.bfloat16}
    dt_qkv = w_dt[MM_QKV]
    dt_proj = w_dt[MM_PROJ]

    in_maps = []
    for ci in range(NCORES):
        c0 = ci * HC
        sl = {}
        for name, off in (("q", 0), ("k", HEADS * DH), ("v", 2 * HEADS * DH)):
            w = w_qkv[:, off + c0: off + c0 + HC]
            sl["w" + name] = np.ascontiguousarray(
                (ln_scale[:, None] * w).astype(dt_qkv)
            )
            sl[name + "b"] = np.ascontiguousarray(ln_bias @ w)
        sl["wo"] = np.ascontiguousarray(w_out[:, c0:c0 + HC].astype(dt_proj))
        sl["bo"] = np.ascontiguousarray(b_out[c0:c0 + HC])
        sl["x"] = x
        in_maps.append(sl)
    return in_maps


def kernel(x, ln_scale, ln_bias, w_qkv, w_out, b_out):
    from concourse.bass_utils import run_bass_kernel_spmd

    nc = _get_built()
    in_maps = _shard_inputs(x, ln_scale, ln_bias, w_qkv, w_out, b_out)
    res = run_bass_kernel_spmd(nc, in_maps, core_ids=list(range(NCORES)))
    shards = [res.results[ci]["out"] for ci in range(NCORES)]  # [128, 2048] each
    outT = np.concatenate(shards, axis=0)  # [1024, 2048]
    return np.ascontiguousarray(outT.T)


# revision 22
# speedup vs baseline: 2.2111x; 1.1175x over previous
"""Distributed Trainium2 kernel for pre-LN multi-head self-attention.

Reference computation (n=2048, d=1024, 16 heads x 64):
    xn  = LayerNorm(x) * ln_scale + ln_bias
    qkv = xn @ w_qkv ; split -> q,k,v [16, 2048, 64]
    sim = (q @ k^T) * d**-0.5 ; attn = softmax(sim)
    out = concat_heads(attn @ v) @ w_out + b_out

Sharding: 2 heads per core (tensor parallel). Each core:
  - computes LayerNorm(x) (replicated) and xn^T via PE transposes
  - projects its 2 heads' q/k/v (ln_scale folded into weights on host,
    ln_bias folded into a per-output-column bias added at PSUM evacuation)
  - attention in transposed layout (keys on partitions) so no transposes
    are needed between the two attention matmuls; a ones-column appended
    to v yields softmax denominators for free
  - AllGather of the per-head outputs (attn_out^T), chunked by row block
  - computes a 128-column slice of the final projection (+ bias)
Host assembles the 8 [128, 2048] outT shards into the [2048, 1024] output.

Scheduling notes: engines execute in emission order, so the code software-
pipelines stage boundaries explicitly — QKV matmuls are emitted inside the
LayerNorm loop per 4-row-tile group, and each (row-chunk, head) stage's
attn@v matmuls are interleaved with the next stage's sim matmuls so the
TensorEngine never idles (keeps the HAM clock gate at 2.4 GHz; a warmup
burst opens it). Projection for a row chunk is deferred ~1.5 stages so its
AllGather completes before the PE reaches those matmuls.
"""

import sys

import ml_dtypes
import numpy as np

for _p in ("/opt/trn_rl_repo", "/root/.axon_site/_ro/trn_rl_repo"):
    if _p not in sys.path:
        sys.path.append(_p)

N = 2048          # sequence length
D = 1024          # model dim
HEADS = 16
DH = 64
NCORES = 8
HL = HEADS // NCORES          # heads per core (2)
HC = HL * DH                  # head cols per core (128)
LN_EPS = 1e-6
SIM_SCALE = float(D) ** -0.5  # reference scales by input dim

P = 128
RT = N // P        # 16 row tiles
DC = D // P        # 8 dim chunks
RC_W = 512         # row-chunk width for attention/collective pipeline
NRC = N // RC_W    # 4 row chunks

# Matmul compute dtype: "bf16" runs the PE in standard mode at full rate
# (and keeps the HAM clock warm - the fp32 modes do not), "f32r" is
# single-pass reduced-precision fp32, "f32" is exact 4-pass fp32.
MM_DT = "bf16"

_BUILT = None


def _build():
    """Build the SPMD Bass graph (same graph on all 8 cores)."""
    from contextlib import ExitStack

    import concourse.tile as tile
    from concourse import bacc, mybir
    from concourse.masks import make_identity

    f32 = mybir.dt.float32
    dt_mm = {"f32": f32, "f32r": mybir.dt.float32r,
             "bf16": mybir.dt.bfloat16}[MM_DT]
    AF = mybir.ActivationFunctionType

    nc = bacc.Bacc(None, num_devices=NCORES)

    x_d = nc.declare_dram_parameter("x", [N, D], f32, isOutput=False)
    wq_d = nc.declare_dram_parameter("wq", [D, HC], dt_mm, isOutput=False)
    wk_d = nc.declare_dram_parameter("wk", [D, HC], dt_mm, isOutput=False)
    wv_d = nc.declare_dram_parameter("wv", [D, HC], dt_mm, isOutput=False)
    qb_d = nc.declare_dram_parameter("qb", [HC], f32, isOutput=False)
    kb_d = nc.declare_dram_parameter("kb", [HC], f32, isOutput=False)
    vb_d = nc.declare_dram_parameter("vb", [HC], f32, isOutput=False)
    wo_d = nc.declare_dram_parameter("wo", [D, HC], dt_mm, isOutput=False)
    bo_d = nc.declare_dram_parameter("bo", [HC], f32, isOutput=False)
    out_d = nc.declare_dram_parameter("out", [HC, N], f32, isOutput=True)

    groups = [list(range(NCORES))]

    with ExitStack() as ctx:
        tc = ctx.enter_context(tile.TileContext(nc))

        dram = ctx.enter_context(tc.tile_pool(name="dram", bufs=1, space="DRAM"))
        ag_in = [dram.tile([HC, RC_W], dt_mm, name=f"ag_in{rc}")
                 for rc in range(NRC)]
        ag_out = [
            dram.tile([NCORES * HC, RC_W], dt_mm, addr_space="Shared",
                      name=f"ag_out{rc}")
            for rc in range(NRC)
        ]

        singles = ctx.enter_context(tc.tile_pool(name="singles", bufs=1))

        ident = singles.tile([P, P], dt_mm)
        make_identity(nc, ident)
        eps_t = singles.tile([P, 1], f32)
        nc.vector.memset(eps_t, LN_EPS)
        ones_t = singles.tile([P, DH], dt_mm)
        nc.vector.memset(ones_t, 1.0)
        warm_rhs = singles.tile([P, RC_W], dt_mm)
        nc.vector.memset(warm_rhs, 0.0)

        # weights / biases
        wq_sb = singles.tile([P, DC, HC], dt_mm)
        wk_sb = singles.tile([P, DC, HC], dt_mm)
        wv_sb = singles.tile([P, DC, HC], dt_mm)
        wo_sb = singles.tile([P, DC, HC], dt_mm)
        for w_sb, w_d in ((wq_sb, wq_d), (wk_sb, wk_d), (wv_sb, wv_d),
                          (wo_sb, wo_d)):
            nc.sync.dma_start(
                out=w_sb, in_=w_d[:, :].rearrange("(c p) m -> p c m", p=P)
            )
        qb_t = singles.tile([P, 1], f32)
        kb_t = singles.tile([P, 1], f32)
        vb_t = singles.tile([P, 1], f32)
        bo_t = singles.tile([P, 1], f32)
        for b_t, b_d in ((qb_t, qb_d), (kb_t, kb_d), (vb_t, vb_d), (bo_t, bo_d)):
            nc.sync.dma_start(out=b_t, in_=b_d[:].rearrange("(p o) -> p o", o=1))

        # long-lived activations
        qT = singles.tile([P, N], dt_mm)        # [2*64 qdims, rows]
        kT = singles.tile([P, N], dt_mm)
        v_sb = singles.tile([P, RT, HL, DH + 1], dt_mm)  # [rowchunk, rt, h, v|1]
        attn_h = [singles.tile([DH, N], dt_mm, name=f"attn_h{h}")
                  for h in range(HL)]
        outT = singles.tile([P, N], f32)

        nc.gpsimd.memset(v_sb[:, :, :, DH:], 1.0)  # ones column

        # ---- stages A-C: LayerNorm -> xn^T -> q/k/v, fused per 4-row group --
        with (
            tc.tile_pool(name="xp", bufs=3) as xp,
            tc.tile_pool(name="stat", bufs=4) as statp,
            tc.tile_pool(name="tp", bufs=2, space="PSUM") as tp,
            tc.tile_pool(name="mmp", bufs=2, space="PSUM") as mmp,
            tc.tile_pool(name="wmp", bufs=1, space="PSUM") as wmp,
            tc.tile_pool(name="xnTp", bufs=1) as xnTp,
        ):
            xnT = xnTp.tile([P, DC, N], dt_mm)   # [dim%128, dimchunk, rows]
            vT = xnTp.tile([P, N], dt_mm)

            # ~4.3us of dependency-free matmuls opens the HAM clock gate
            warm_ps = wmp.tile([P, RC_W], f32, tag="warm")
            for _ in range(10):
                nc.tensor.matmul(warm_ps, ident, warm_rhs,
                                 start=True, stop=True)

            for g4 in range(RT // 4):
                for rt in range(g4 * 4, g4 * 4 + 4):
                    x_t = xp.tile([P, D], f32, tag="x")
                    nc.sync.dma_start(out=x_t, in_=x_d[rt * P:(rt + 1) * P, :])
                    stats = statp.tile([P, 2, 6], f32, tag="st")
                    for sg in range(2):
                        nc.vector.bn_stats(
                            out=stats[:, sg, :],
                            in_=x_t[:, sg * 512:(sg + 1) * 512],
                        )
                    mv = statp.tile([P, 2], f32, tag="mv")
                    nc.vector.bn_aggr(out=mv, in_=stats)
                    rstd = statp.tile([P, 1], f32, tag="rstd")
                    nc.scalar.activation(
                        out=rstd, in_=mv[:, 1:2], func=AF.Sqrt,
                        bias=eps_t, scale=1.0,
                    )
                    nc.vector.reciprocal(out=rstd, in_=rstd)
                    xh_t = xp.tile([P, D], dt_mm, tag="xh")
                    nc.vector.tensor_scalar(
                        out=xh_t, in0=x_t,
                        scalar1=mv[:, 0:1], scalar2=rstd,
                        op0=mybir.AluOpType.subtract, op1=mybir.AluOpType.mult,
                    )
                    # transpose row tile into xnT: 8 [128,128] PE transposes,
                    # 4 per PSUM bank pair; evacuation alternates DVE/ACT
                    for g in range(2):
                        pt = tp.tile([P, 512], dt_mm, tag="pt")
                        with nc.allow_low_precision(reason="transpose copy"):
                            for j in range(4):
                                dc = g * 4 + j
                                nc.tensor.transpose(
                                    pt[:, j * P:(j + 1) * P],
                                    xh_t[:, dc * P:(dc + 1) * P],
                                    ident,
                                )
                        dst = xnT[:, g * 4:(g + 1) * 4, rt * P:(rt + 1) * P]
                        src = pt[:].rearrange("p (j q) -> p j q", j=4)
                        if (rt + g) % 2 == 0:
                            nc.vector.tensor_copy(out=dst, in_=src)
                        else:
                            nc.scalar.copy(out=dst, in_=src)

                # q/k/v projections for this 512-row block
                nt = g4
                for w_sb, b_t, dst in (
                    (wq_sb, qb_t, qT), (wk_sb, kb_t, kT), (wv_sb, vb_t, vT)
                ):
                    pm = mmp.tile([P, 512], f32, tag="pm")
                    for kc in range(DC):
                        nc.tensor.matmul(
                            pm,
                            w_sb[:, kc, :],
                            xnT[:, kc, nt * 512:(nt + 1) * 512],
                            start=(kc == 0), stop=(kc == DC - 1),
                        )
                    nc.scalar.activation(
                        out=dst[:, nt * 512:(nt + 1) * 512], in_=pm,
                        func=AF.Identity, bias=b_t, scale=1.0,
                    )
                # v^T -> v (row-major with ones column) for this block
                for rt in range(g4 * 4, g4 * 4 + 4):
                    pt = tp.tile([P, 512], dt_mm, tag="pt")
                    with nc.allow_low_precision(reason="transpose copy"):
                        nc.tensor.transpose(
                            pt[:, :P], vT[:, rt * P:(rt + 1) * P], ident
                        )
                    nc.vector.tensor_copy(
                        out=v_sb[:, rt, :, 0:DH],
                        in_=pt[:, :P].rearrange("p (h d) -> p h d", h=HL),
                    )

        # ---- stage D: attention, software-pipelined across (rc, h) stages --
        stages = [(rc, h) for rc in range(NRC) for h in range(HL)]
        S = len(stages)

        with (
            tc.tile_pool(name="expp", bufs=2) as expp,
            tc.tile_pool(name="rsum", bufs=4) as rsump,
            tc.tile_pool(name="sp", bufs=2, space="PSUM") as sp,
            tc.tile_pool(name="op", bufs=2, space="PSUM") as op,
            tc.tile_pool(name="rp", bufs=1, space="PSUM") as rp,
            tc.tile_pool(name="agp", bufs=2) as agp,
            tc.tile_pool(name="fp", bufs=1, space="PSUM") as fp,
        ):
            state = {}

            def sim_group(idx, g):
                """Two sim matmuls (key chunks 2g, 2g+1) + one batched exp."""
                rc, h = stages[idx]
                hp, r0 = h * DH, rc * RC_W
                st = state[idx]
                ps = sp.tile([P, 2 * RC_W], f32, tag="ps")
                for j in (0, 1):
                    kc = 2 * g + j
                    nc.tensor.matmul(
                        ps[:, j * RC_W:(j + 1) * RC_W],
                        kT[hp:hp + DH, kc * P:(kc + 1) * P],
                        qT[hp:hp + DH, r0:r0 + RC_W],
                        start=True, stop=True,
                    )
                nc.scalar.activation(
                    out=st["exp_t"][:, 2 * g:2 * g + 2, :],
                    in_=ps[:].rearrange("p (c w) -> p c w", c=2),
                    func=AF.Exp, scale=SIM_SCALE,
                )

            def av_pair(idx, g):
                rc, h = stages[idx]
                st = state[idx]
                for j in (0, 1):
                    kc = 2 * g + j
                    nc.tensor.matmul(
                        st["po"][0:DH + 1, :],
                        v_sb[:, kc, h, :],
                        st["exp_t"][:, kc, :],
                        start=(kc == 0), stop=(kc == RT - 1),
                    )

            def norm_tail(idx):
                """Normalize by softmax denominators, ship to the AG buffer."""
                rc, h = stages[idx]
                hp, r0 = h * DH, rc * RC_W
                st = state[idx]
                po = st["po"]
                rs = rsump.tile([P, RC_W], dt_mm, tag="rs")
                with nc.allow_low_precision(reason="softmax recips"):
                    nc.vector.reciprocal(
                        out=rs[DH:DH + 1, :], in_=po[DH:DH + 1, :]
                    )
                pr = rp.tile([DH, RC_W], f32, tag="pr")
                nc.tensor.matmul(
                    pr, ones_t[DH:DH + 1, :], rs[DH:DH + 1, :],
                    start=True, stop=True,
                )
                rb = rsump.tile([DH, RC_W], f32, tag="rb")
                nc.vector.tensor_copy(out=rb, in_=pr)
                with nc.allow_low_precision(reason="attn bf16 wire"):
                    nc.vector.tensor_mul(
                        out=attn_h[h][:, r0:r0 + RC_W],
                        in0=po[0:DH, :], in1=rb,
                    )
                nc.sync.dma_start(
                    out=ag_in[rc][hp:hp + DH, :],
                    in_=attn_h[h][:, r0:r0 + RC_W],
                )
                if h == HL - 1:
                    nc.gpsimd.collective_compute(
                        "AllGather",
                        mybir.AluOpType.bypass,
                        replica_groups=groups,
                        ins=[ag_in[rc][:].opt()],
                        outs=[ag_out[rc][:].opt()],
                    )

            def proj(rc):
                """outT slice for this row chunk from the gathered heads."""
                r0 = rc * RC_W
                agt = agp.tile([P, DC, RC_W], dt_mm, tag="agt")
                for kc in range(DC):
                    nc.sync.dma_start(
                        out=agt[:, kc, :],
                        in_=ag_out[rc][kc * P:(kc + 1) * P, :],
                    )
                pf = fp.tile([P, RC_W], f32, tag="pf")
                for kc in range(DC):
                    nc.tensor.matmul(
                        pf,
                        wo_sb[:, kc, :],
                        agt[:, kc, :],
                        start=(kc == 0), stop=(kc == DC - 1),
                    )
                nc.scalar.activation(
                    out=outT[:, r0:r0 + RC_W], in_=pf,
                    func=AF.Identity, bias=bo_t, scale=1.0,
                )
                nc.sync.dma_start(
                    out=out_d[:, r0:r0 + RC_W], in_=outT[:, r0:r0 + RC_W]
                )

            pending_proj = []
            for idx in range(S):
                state[idx] = {
                    "exp_t": expp.tile([P, RT, RC_W], dt_mm, tag="exp",
                                       name=f"exp{idx}"),
                    "po": op.tile([P, RC_W], f32, tag="po", name=f"po{idx}"),
                }
                for g in range(RT // 2):
                    sim_group(idx, g)
                    if idx > 0:
                        av_pair(idx - 1, g)
                    if g == 3 and pending_proj:
                        proj(pending_proj.pop(0))
                if idx > 0:
                    norm_tail(idx - 1)
                    rc_prev, h_prev = stages[idx - 1]
                    if h_prev == HL - 1:
                        pending_proj.append(rc_prev)
            # drain: last stage's av + norm + its AG + remaining projections
            for g in range(RT // 2):
                av_pair(S - 1, g)
            norm_tail(S - 1)
            pending_proj.append(NRC - 1)
            for rc in pending_proj:
                proj(rc)

    if not nc.is_finalized():
        nc.finalize()
    return nc


def _get_built():
    global _BUILT
    if _BUILT is None:
        _BUILT = _build()
    return _BUILT


def _shard_inputs(x, ln_scale, ln_bias, w_qkv, w_out, b_out):
    """Host-side sharding: slice per-head weight columns, fold LN params."""
    x = np.ascontiguousarray(np.asarray(x, np.float32))
    ln_scale = np.asarray(ln_scale, np.float32)
    ln_bias = np.asarray(ln_bias, np.float32)
    w_qkv = np.asarray(w_qkv, np.float32)
    w_out = np.asarray(w_out, np.float32)
    b_out = np.asarray(b_out, np.float32)

    w_np = {"f32": np.float32, "f32r": np.float32,
            "bf16": ml_dtypes.bfloat16}[MM_DT]

    in_maps = []
    for ci in range(NCORES):
        c0 = ci * HC
        sl = {}
        for name, off in (("q", 0), ("k", HEADS * DH), ("v", 2 * HEADS * DH)):
            w = w_qkv[:, off + c0: off + c0 + HC]
            sl["w" + name] = np.ascontiguousarray(
                (ln_scale[:, None] * w).astype(w_np)
            )
            sl[name + "b"] = np.ascontiguousarray(ln_bias @ w)
        sl["wo"] = np.ascontiguousarray(w_out[:, c0:c0 + HC].astype(w_np))
        sl["bo"] = np.ascontiguousarray(b_out[c0:c0 + HC])
        sl["x"] = x
        in_maps.append(sl)
    return in_maps


def kernel(x, ln_scale, ln_bias, w_qkv, w_out, b_out):
    from concourse.bass_utils import run_bass_kernel_spmd

    nc = _get_built()
    in_maps = _shard_inputs(x, ln_scale, ln_bias, w_qkv, w_out, b_out)
    res = run_bass_kernel_spmd(nc, in_maps, core_ids=list(range(NCORES)))
    shards = [res.results[ci]["out"] for ci in range(NCORES)]  # [128, 2048] each
    outT = np.concatenate(shards, axis=0)  # [1024, 2048]
    return np.ascontiguousarray(outT.T)


# revision 24
# speedup vs baseline: 2.5285x; 1.1436x over previous
"""Distributed Trainium2 kernel for pre-LN multi-head self-attention.

Reference computation (n=2048, d=1024, 16 heads x 64):
    xn  = LayerNorm(x) * ln_scale + ln_bias
    qkv = xn @ w_qkv ; split -> q,k,v [16, 2048, 64]
    sim = (q @ k^T) * d**-0.5 ; attn = softmax(sim)
    out = concat_heads(attn @ v) @ w_out + b_out

Sharding: 2 heads per core (tensor parallel). Each core:
  - computes LayerNorm(x) (replicated) and xn^T via PE transposes
  - projects its 2 heads' q/k/v (ln_scale folded into weights on host,
    ln_bias folded into a per-output-column bias added at PSUM evacuation)
  - attention in transposed layout (keys on partitions) so no transposes
    are needed between the two attention matmuls; a ones-column appended
    to v yields softmax denominators for free
  - AllGather of the per-head outputs (attn_out^T), chunked by row block
  - computes a 128-column slice of the final projection (+ bias)
Host assembles the 8 [128, 2048] outT shards into the [2048, 1024] output.

Scheduling notes: engines execute in emission order, so the code software-
pipelines stage boundaries explicitly — QKV matmuls are emitted inside the
LayerNorm loop per 4-row-tile group, and each (row-chunk, head) stage's
attn@v matmuls are interleaved with the next stage's sim matmuls so the
TensorEngine never idles (keeps the HAM clock gate at 2.4 GHz; a warmup
burst opens it). Projection for a row chunk is deferred ~1.5 stages so its
AllGather completes before the PE reaches those matmuls.
"""

import sys

import ml_dtypes
import numpy as np

for _p in ("/opt/trn_rl_repo", "/root/.axon_site/_ro/trn_rl_repo"):
    if _p not in sys.path:
        sys.path.append(_p)

N = 2048          # sequence length
D = 1024          # model dim
HEADS = 16
DH = 64
NCORES = 8
HL = HEADS // NCORES          # heads per core (2)
HC = HL * DH                  # head cols per core (128)
LN_EPS = 1e-6
SIM_SCALE = float(D) ** -0.5  # reference scales by input dim

P = 128
RT = N // P        # 16 row tiles
DC = D // P        # 8 dim chunks
RC_W = 512         # row-chunk width for attention/collective pipeline
NRC = N // RC_W    # 4 row chunks

# Matmul compute dtype: "bf16" runs the PE in standard mode at full rate
# (and keeps the HAM clock warm - the fp32 modes do not), "f32r" is
# single-pass reduced-precision fp32, "f32" is exact 4-pass fp32.
MM_DT = "bf16"

_BUILT = None


def _build():
    """Build the SPMD Bass graph (same graph on all 8 cores)."""
    from contextlib import ExitStack

    import concourse.tile as tile
    from concourse import bacc, mybir
    from concourse.masks import make_identity

    f32 = mybir.dt.float32
    dt_mm = {"f32": f32, "f32r": mybir.dt.float32r,
             "bf16": mybir.dt.bfloat16}[MM_DT]
    AF = mybir.ActivationFunctionType

    nc = bacc.Bacc(None, num_devices=NCORES)

    x_d = nc.declare_dram_parameter("x", [N, D], f32, isOutput=False)
    wq_d = nc.declare_dram_parameter("wq", [D, HC], dt_mm, isOutput=False)
    wk_d = nc.declare_dram_parameter("wk", [D, HC], dt_mm, isOutput=False)
    wv_d = nc.declare_dram_parameter("wv", [D, HC], dt_mm, isOutput=False)
    qb_d = nc.declare_dram_parameter("qb", [HC], f32, isOutput=False)
    kb_d = nc.declare_dram_parameter("kb", [HC], f32, isOutput=False)
    vb_d = nc.declare_dram_parameter("vb", [HC], f32, isOutput=False)
    wo_d = nc.declare_dram_parameter("wo", [D, HC], dt_mm, isOutput=False)
    bo_d = nc.declare_dram_parameter("bo", [HC], f32, isOutput=False)
    out_d = nc.declare_dram_parameter("out", [HC, N], f32, isOutput=True)

    groups = [list(range(NCORES))]

    with ExitStack() as ctx:
        tc = ctx.enter_context(tile.TileContext(nc))

        dram = ctx.enter_context(tc.tile_pool(name="dram", bufs=1, space="DRAM"))
        ag_widths = [512, 512, 512, 256, 256]
        ag_in = [dram.tile([HC, w], dt_mm, name=f"ag_in{i}")
                 for i, w in enumerate(ag_widths)]
        ag_out = [
            dram.tile([NCORES * HC, w], dt_mm, addr_space="Shared",
                      name=f"ag_out{i}")
            for i, w in enumerate(ag_widths)
        ]

        singles = ctx.enter_context(tc.tile_pool(name="singles", bufs=1))

        ident = singles.tile([P, P], dt_mm)
        make_identity(nc, ident)
        eps_t = singles.tile([P, 1], f32)
        nc.vector.memset(eps_t, LN_EPS)
        ones_t = singles.tile([P, DH], dt_mm)
        nc.vector.memset(ones_t, 1.0)
        warm_rhs = singles.tile([P, RC_W], dt_mm)
        nc.vector.memset(warm_rhs, 0.0)

        # weights / biases
        wq_sb = singles.tile([P, DC, HC], dt_mm)
        wk_sb = singles.tile([P, DC, HC], dt_mm)
        wv_sb = singles.tile([P, DC, HC], dt_mm)
        wo_sb = singles.tile([P, DC, HC], dt_mm)
        for w_sb, w_d in ((wq_sb, wq_d), (wk_sb, wk_d), (wv_sb, wv_d),
                          (wo_sb, wo_d)):
            nc.sync.dma_start(
                out=w_sb, in_=w_d[:, :].rearrange("(c p) m -> p c m", p=P)
            )
        qb_t = singles.tile([P, 1], f32)
        kb_t = singles.tile([P, 1], f32)
        vb_t = singles.tile([P, 1], f32)
        bo_t = singles.tile([P, 1], f32)
        for b_t, b_d in ((qb_t, qb_d), (kb_t, kb_d), (vb_t, vb_d), (bo_t, bo_d)):
            nc.sync.dma_start(out=b_t, in_=b_d[:].rearrange("(p o) -> p o", o=1))

        # long-lived activations
        qT = singles.tile([P, N], dt_mm)        # [2*64 qdims, rows]
        kT = singles.tile([P, N], dt_mm)
        v_sb = singles.tile([P, RT, HL, DH + 1], dt_mm)  # [rowchunk, rt, h, v|1]
        attn_h = [singles.tile([DH, N], dt_mm, name=f"attn_h{h}")
                  for h in range(HL)]
        outT = singles.tile([P, N], f32)

        nc.gpsimd.memset(v_sb[:, :, :, DH:], 1.0)  # ones column

        # ---- stages A-C: LayerNorm -> xn^T -> q/k/v, fused per 4-row group --
        with (
            tc.tile_pool(name="xp", bufs=3) as xp,
            tc.tile_pool(name="stat", bufs=4) as statp,
            tc.tile_pool(name="tp", bufs=2, space="PSUM") as tp,
            tc.tile_pool(name="mmp", bufs=2, space="PSUM") as mmp,
            tc.tile_pool(name="wmp", bufs=1, space="PSUM") as wmp,
            tc.tile_pool(name="xnTp", bufs=1) as xnTp,
        ):
            xnT = xnTp.tile([P, DC, N], dt_mm)   # [dim%128, dimchunk, rows]
            vT = xnTp.tile([P, N], dt_mm)

            # ~4.3us of dependency-free matmuls opens the HAM clock gate
            warm_ps = wmp.tile([P, RC_W], f32, tag="warm")
            for _ in range(10):
                nc.tensor.matmul(warm_ps, ident, warm_rhs,
                                 start=True, stop=True)

            for g4 in range(RT // 4):
                for rt in range(g4 * 4, g4 * 4 + 4):
                    x_t = xp.tile([P, D], f32, tag="x")
                    nc.sync.dma_start(out=x_t, in_=x_d[rt * P:(rt + 1) * P, :])
                    stats = statp.tile([P, 2, 6], f32, tag="st")
                    for sg in range(2):
                        nc.vector.bn_stats(
                            out=stats[:, sg, :],
                            in_=x_t[:, sg * 512:(sg + 1) * 512],
                        )
                    mv = statp.tile([P, 2], f32, tag="mv")
                    nc.vector.bn_aggr(out=mv, in_=stats)
                    rstd = statp.tile([P, 1], f32, tag="rstd")
                    nc.scalar.activation(
                        out=rstd, in_=mv[:, 1:2], func=AF.Sqrt,
                        bias=eps_t, scale=1.0,
                    )
                    nc.vector.reciprocal(out=rstd, in_=rstd)
                    xh_t = xp.tile([P, D], dt_mm, tag="xh")
                    nc.vector.tensor_scalar(
                        out=xh_t, in0=x_t,
                        scalar1=mv[:, 0:1], scalar2=rstd,
                        op0=mybir.AluOpType.subtract, op1=mybir.AluOpType.mult,
                    )
                    # transpose row tile into xnT: 8 [128,128] PE transposes,
                    # 4 per PSUM bank pair; evacuation alternates DVE/ACT
                    for g in range(2):
                        pt = tp.tile([P, 512], dt_mm, tag="pt")
                        with nc.allow_low_precision(reason="transpose copy"):
                            for j in range(4):
                                dc = g * 4 + j
                                nc.tensor.transpose(
                                    pt[:, j * P:(j + 1) * P],
                                    xh_t[:, dc * P:(dc + 1) * P],
                                    ident,
                                )
                        dst = xnT[:, g * 4:(g + 1) * 4, rt * P:(rt + 1) * P]
                        src = pt[:].rearrange("p (j q) -> p j q", j=4)
                        if (rt + g) % 2 == 0:
                            nc.vector.tensor_copy(out=dst, in_=src)
                        else:
                            nc.scalar.copy(out=dst, in_=src)

                # q/k/v projections for this 512-row block
                nt = g4
                for w_sb, b_t, dst in (
                    (wq_sb, qb_t, qT), (wk_sb, kb_t, kT), (wv_sb, vb_t, vT)
                ):
                    pm = mmp.tile([P, 512], f32, tag="pm")
                    for kc in range(DC):
                        nc.tensor.matmul(
                            pm,
                            w_sb[:, kc, :],
                            xnT[:, kc, nt * 512:(nt + 1) * 512],
                            start=(kc == 0), stop=(kc == DC - 1),
                        )
                    nc.scalar.activation(
                        out=dst[:, nt * 512:(nt + 1) * 512], in_=pm,
                        func=AF.Identity, bias=b_t, scale=1.0,
                    )
                # v^T -> v (row-major with ones column) for this block
                for rt in range(g4 * 4, g4 * 4 + 4):
                    pt = tp.tile([P, 512], dt_mm, tag="pt")
                    with nc.allow_low_precision(reason="transpose copy"):
                        nc.tensor.transpose(
                            pt[:, :P], vT[:, rt * P:(rt + 1) * P], ident
                        )
                    nc.vector.tensor_copy(
                        out=v_sb[:, rt, :, 0:DH],
                        in_=pt[:, :P].rearrange("p (h d) -> p h d", h=HL),
                    )

        # ---- stage D: attention, software-pipelined across row-chunk stages --
        # Each stage covers one row chunk with BOTH heads: the two heads' sim
        # matmuls pack into disjoint row groups of the PE array (partitions
        # 0-63 / 64-127) and run concurrently; attn@v for the two heads
        # alternates between two PSUM banks, so consecutive accumulating
        # matmuls never serialize on one bank. A stage's attn@v is emitted
        # interleaved with the NEXT stage's sim so the PE stream never stalls
        # on the exp (ACT) producer. The trailing chunks are 256 rows wide so
        # the final (unoverlappable) AllGather + projection tail is small.
        chunks = [(0, 512), (512, 512), (1024, 512), (1536, 256), (1792, 256)]
        S = len(chunks)

        with (
            tc.tile_pool(name="expp", bufs=2) as expp,
            tc.tile_pool(name="rsum", bufs=4) as rsump,
            tc.tile_pool(name="sp", bufs=2, space="PSUM") as sp,
            tc.tile_pool(name="op", bufs=1, space="PSUM") as op,
            tc.tile_pool(name="rp", bufs=1, space="PSUM") as rp,
            tc.tile_pool(name="agp", bufs=2) as agp,
            tc.tile_pool(name="fp", bufs=1, space="PSUM") as fp,
        ):
            state = {}

            def sim_group(idx, kc):
                """Both heads' sim for one key chunk, row-group packed."""
                r0, w = chunks[idx]
                st = state[idx]
                ps = sp.tile([P, 2 * RC_W], f32, tag="ps", name=f"ps{idx}_{kc}")
                for h in range(HL):
                    nc.tensor.matmul(
                        ps[:, h * RC_W:h * RC_W + w],
                        kT[h * DH:(h + 1) * DH, kc * P:(kc + 1) * P],
                        qT[h * DH:(h + 1) * DH, r0:r0 + w],
                        start=True, stop=True,
                    )
                nc.scalar.activation(
                    out=st["exp_t"][:, kc, :, 0:w],
                    in_=ps[:].rearrange("p (c q) -> p c q", c=2)[:, :, 0:w],
                    func=AF.Exp, scale=SIM_SCALE,
                )

            def av_pair(idx, kc):
                """attn@v for key chunk kc, both heads (alternating banks)."""
                r0, w = chunks[idx]
                st = state[idx]
                if st["po"] is None:
                    st["po"] = op.tile([P, 2 * RC_W], f32, tag="po",
                                       name=f"po{idx}")
                for h in range(HL):
                    nc.tensor.matmul(
                        st["po"][0:DH + 1, h * RC_W:h * RC_W + w],
                        v_sb[:, kc, h, :],
                        st["exp_t"][:, kc, h, 0:w],
                        start=(kc == 0), stop=(kc == RT - 1),
                    )

            def norm_tail(idx):
                """Normalize by softmax denominators, ship to the AG buffer."""
                r0, w = chunks[idx]
                st = state[idx]
                po = st["po"]
                for h in range(HL):
                    ph = po[:, h * RC_W:h * RC_W + w]
                    rs = rsump.tile([P, RC_W], dt_mm, tag="rs",
                                    name=f"rs{idx}_{h}")
                    with nc.allow_low_precision(reason="softmax recips"):
                        nc.vector.reciprocal(
                            out=rs[DH:DH + 1, 0:w], in_=ph[DH:DH + 1, :]
                        )
                    pr = rp.tile([DH, RC_W], f32, tag="pr", name=f"pr{idx}_{h}")
                    nc.tensor.matmul(
                        pr[:, 0:w], ones_t[DH:DH + 1, :], rs[DH:DH + 1, 0:w],
                        start=True, stop=True,
                    )
                    rb = rsump.tile([DH, RC_W], f32, tag="rb",
                                    name=f"rb{idx}_{h}")
                    nc.vector.tensor_copy(out=rb[:, 0:w], in_=pr[:, 0:w])
                    with nc.allow_low_precision(reason="attn bf16 wire"):
                        nc.vector.tensor_mul(
                            out=attn_h[h][:, r0:r0 + w],
                            in0=ph[0:DH, :], in1=rb[:, 0:w],
                        )
                    nc.sync.dma_start(
                        out=ag_in[idx][h * DH:(h + 1) * DH, :],
                        in_=attn_h[h][:, r0:r0 + w],
                    )
                nc.gpsimd.collective_compute(
                    "AllGather",
                    mybir.AluOpType.bypass,
                    replica_groups=groups,
                    ins=[ag_in[idx][:].opt()],
                    outs=[ag_out[idx][:].opt()],
                )

            def proj(idx):
                """outT slice for this row chunk from the gathered heads."""
                r0, w = chunks[idx]
                agt = agp.tile([P, DC, RC_W], dt_mm, tag="agt",
                               name=f"agt{idx}")
                for kc in range(DC):
                    nc.sync.dma_start(
                        out=agt[:, kc, 0:w],
                        in_=ag_out[idx][kc * P:(kc + 1) * P, :],
                    )
                pf = fp.tile([P, RC_W], f32, tag="pf", name=f"pf{idx}")
                for kc in range(DC):
                    nc.tensor.matmul(
                        pf[:, 0:w],
                        wo_sb[:, kc, :],
                        agt[:, kc, 0:w],
                        start=(kc == 0), stop=(kc == DC - 1),
                    )
                nc.scalar.activation(
                    out=outT[:, r0:r0 + w], in_=pf[:, 0:w],
                    func=AF.Identity, bias=bo_t, scale=1.0,
                )
                nc.sync.dma_start(
                    out=out_d[:, r0:r0 + w], in_=outT[:, r0:r0 + w]
                )

            pending_proj = []
            for idx in range(S):
                state[idx] = {
                    "exp_t": expp.tile([P, RT, HL, RC_W], dt_mm, tag="exp",
                                       name=f"exp{idx}"),
                    "po": None,
                }
                for kc in range(RT):
                    sim_group(idx, kc)
                    if idx > 0:
                        av_pair(idx - 1, kc)
                    if kc == 7 and pending_proj:
                        proj(pending_proj.pop(0))
                if idx > 0:
                    norm_tail(idx - 1)
                    pending_proj.append(idx - 1)
            # drain: last stage's av + norm + AG + remaining projections
            for kc in range(RT):
                av_pair(S - 1, kc)
            # remaining projections before the last AG so they overlap it
            norm_tail(S - 1)
            for idx in pending_proj:
                proj(idx)
            proj(S - 1)

    if not nc.is_finalized():
        nc.finalize()
    return nc


def _get_built():
    global _BUILT
    if _BUILT is None:
        _BUILT = _build()
    return _BUILT


def _shard_inputs(x, ln_scale, ln_bias, w_qkv, w_out, b_out):
    """Host-side sharding: slice per-head weight columns, fold LN params."""
    x = np.ascontiguousarray(np.asarray(x, np.float32))
    ln_scale = np.asarray(ln_scale, np.float32)
    ln_bias = np.asarray(ln_bias, np.float32)
    w_qkv = np.asarray(w_qkv, np.float32)
    w_out = np.asarray(w_out, np.float32)
    b_out = np.asarray(b_out, np.float32)

    w_np = {"f32": np.float32, "f32r": np.float32,
            "bf16": ml_dtypes.bfloat16}[MM_DT]

    in_maps = []
    for ci in range(NCORES):
        c0 = ci * HC
        sl = {}
        for name, off in (("q", 0), ("k", HEADS * DH), ("v", 2 * HEADS * DH)):
            w = w_qkv[:, off + c0: off + c0 + HC]
            sl["w" + name] = np.ascontiguousarray(
                (ln_scale[:, None] * w).astype(w_np)
            )
            sl[name + "b"] = np.ascontiguousarray(ln_bias @ w)
        sl["wo"] = np.ascontiguousarray(w_out[:, c0:c0 + HC].astype(w_np))
        sl["bo"] = np.ascontiguousarray(b_out[c0:c0 + HC])
        sl["x"] = x
        in_maps.append(sl)
    return in_maps


def kernel(x, ln_scale, ln_bias, w_qkv, w_out, b_out):
    from concourse.bass_utils import run_bass_kernel_spmd

    nc = _get_built()
    in_maps = _shard_inputs(x, ln_scale, ln_bias, w_qkv, w_out, b_out)
    res = run_bass_kernel_spmd(nc, in_maps, core_ids=list(range(NCORES)))
    shards = [res.results[ci]["out"] for ci in range(NCORES)]  # [128, 2048] each
    outT = np.concatenate(shards, axis=0)  # [1024, 2048]
    return np.ascontiguousarray(outT.T)
